# revision 5
# baseline (speedup 1.0000x reference)
"""Trainium2 Bass kernel for staircase-sparse varlen GQA attention + paged KV-cache store.

Problem (hardcoded shapes):
  q [8192,16,128] f32, k/v [8192,4,128] f32, k_cache/v_cache [16384,4,128] f32,
  slot_mapping arange(8192) i32, cu_seqlens arange(9)*1024 i32, block_size=128.
Returns (o [8192,2048] f32, k_cache_new, v_cache_new).

Sharding: data-parallel over the B=8 sequences (one per NeuronCore); the
KV-cache scatter + untouched-region copy is sharded over cores as well.

Per-core device kernel:
  - q/k loaded with an f32->f16 casting DMA (SWDGE), transposed on TensorE
    (identity matmul) into [d, token] layout.
  - S^T[kpos, q] = kT_j^T . qT  on TensorE (fp16 in, fp32 PSUM out).
  - p^T = exp(SCALE * S^T) on ScalarE (PSUM f32 -> SBUF f16), batched wide.
  - o[q, 0:128] and softmax denominator o[q, 128] accumulate in one matmul:
    lhsT = p^T, rhs = [v_j | ones]  (ones-column trick), PSUM accumulation
    over the staircase j<=i.
  - normalize with DVE reciprocal + tensor_scalar_mul, DMA out.
  - cache update: DRAM->DRAM DMA copies (touched slots from k/v inputs,
    untouched slots passed through).
"""

import numpy as np

import concourse.bass as bass
import concourse.mybir as mybir
import concourse.tile as tile
from concourse import bacc, bass_utils
from concourse.masks import make_identity

# ---- problem constants (hardcoded per harness contract) ----
B, S, H, KV, D = 8, 1024, 16, 4, 128
T = B * S
NUM_SLOTS = 16384
BLOCK = 128
NBLK = S // BLOCK          # 8 staircase blocks per sequence
G = H // KV                # 4 query heads per kv head
SCALE = 0.08838834764831845
N_CORES = 8
UNTOUCHED = NUM_SLOTS - T          # 8192 slots keep their old cache value
UN_PER_CORE = UNTOUCHED // N_CORES  # 1024
VST = 132                  # vb column stride per (kv, j): 128 v cols + 1 ones + pad

F32 = mybir.dt.float32
F16 = mybir.dt.float16

def _emit(nc, tc):
    q_d = nc.dram_tensor("q", [S, H, D], F32, kind="ExternalInput").ap()
    k_d = nc.dram_tensor("k", [S, KV, D], F32, kind="ExternalInput").ap()
    v_d = nc.dram_tensor("v", [S, KV, D], F32, kind="ExternalInput").ap()
    kcu_d = nc.dram_tensor("kc_un", [UN_PER_CORE, KV, D], F32, kind="ExternalInput").ap()
    vcu_d = nc.dram_tensor("vc_un", [UN_PER_CORE, KV, D], F32, kind="ExternalInput").ap()
    o_d = nc.dram_tensor("o", [S, H * D], F32, kind="ExternalOutput").ap()
    kco_d = nc.dram_tensor("kc_out", [S + UN_PER_CORE, KV, D], F32, kind="ExternalOutput").ap()
    vco_d = nc.dram_tensor("vc_out", [S + UN_PER_CORE, KV, D], F32, kind="ExternalOutput").ap()

    # cache copy jobs, chunked so they spread across the kernel
    cache_jobs = []
    n_chunks = 2
    rows = S // n_chunks
    for dst, src, base in ((kco_d, k_d, 0), (kco_d, kcu_d, S),
                           (vco_d, v_d, 0), (vco_d, vcu_d, S)):
        for c in range(n_chunks):
            cache_jobs.append((dst[base + c * rows: base + (c + 1) * rows],
                               src[c * rows: (c + 1) * rows]))

    with (
        tc.tile_pool(name="pers", bufs=1) as pers,
        tc.tile_pool(name="nat", bufs=3) as nat_pool,
        tc.tile_pool(name="pt", bufs=2) as pt_pool,
        tc.tile_pool(name="osb", bufs=4) as osb_pool,
        tc.tile_pool(name="small", bufs=4) as small_pool,
    ):
        ident = pers.tile([128, 128], F16, tag="ident")
        make_identity(nc, ident[:])

        # qT is i-block-major: column index = (i*H + h)*128 + p, so that for a
        # fixed q-block i the heads are contiguous (lets S^T matmuls span 2
        # heads, N=256).
        qT = pers.tile([128, H * S], F16, tag="qT")
        kT = pers.tile([128, KV * S], F16, tag="kT")
        vb = pers.tile([128, KV * NBLK * VST], F16, tag="vb")

        # ones columns of vb (position 128 in each VST-stride slot)
        vb3 = vb[:].rearrange("p (n x) -> p n x", x=VST)
        nc.vector.memset(vb3[:, :, D:D + 1], 1.0)

        with tc.tile_pool(name="tp", bufs=4, space="PSUM") as tp_pool:
            # ---- k: load (cast f32->f16) + transpose ----
            for kv in range(KV):
                knat = nat_pool.tile([128, NBLK * D], F16, tag="nat")
                knat3 = knat[:].rearrange("p (j d) -> p j d", j=NBLK)
                nc.gpsimd.dma_start(knat3, k_d[:, kv, :].rearrange("(j p) d -> p j d", p=128))
                for j in range(NBLK):
                    tp = tp_pool.tile([128, 128], F16, tag="tp")
                    nc.tensor.transpose(tp[:], knat[:, j * D:(j + 1) * D], ident[:])
                    nc.vector.tensor_copy(kT[:, kv * S + j * BLOCK: kv * S + (j + 1) * BLOCK], tp[:])

            # ---- v: load with cast directly into vb slots ----
            for kv in range(KV):
                dst = vb[:, kv * NBLK * VST: (kv + 1) * NBLK * VST]
                dst3 = dst.rearrange("p (j x) -> p j x", j=NBLK)[:, :, 0:D]
                nc.gpsimd.dma_start(dst3, v_d[:, kv, :].rearrange("(j p) d -> p j d", p=128))

            # ---- q: load (cast) + transpose ----
            for h in range(H):
                qnat = nat_pool.tile([128, NBLK * D], F16, tag="nat")
                qnat3 = qnat[:].rearrange("p (i d) -> p i d", i=NBLK)
                nc.gpsimd.dma_start(qnat3, q_d[:, h, :].rearrange("(i p) d -> p i d", p=128))
                for i in range(NBLK):
                    tp = tp_pool.tile([128, 128], F16, tag="tp")
                    nc.tensor.transpose(tp[:], qnat[:, i * D:(i + 1) * D], ident[:])
                    nc.vector.tensor_copy(
                        qT[:, (i * H + h) * BLOCK: (i * H + h + 1) * BLOCK], tp[:])

        # ---- attention: 2 heads per pass, i-outer, j-inner ----
        job_idx = 0
        with (
            tc.tile_pool(name="stp", bufs=1, space="PSUM") as st_pool,
            tc.tile_pool(name="oap", bufs=4, space="PSUM") as oacc_pool,
        ):
            for kv in range(KV):
                for hp in range(G // 2):
                    h0 = kv * G + hp * 2  # heads h0, h0+1
                    for i in range(NBLK):
                        ncols = 2 * BLOCK
                        st = st_pool.tile([128, 2048], F32, tag="st")
                        pt = pt_pool.tile([128, 2048], F16, tag="pt")
                        for j in range(i + 1):
                            nc.tensor.matmul(
                                st[:, j * ncols: (j + 1) * ncols],
                                lhsT=kT[:, kv * S + j * BLOCK: kv * S + (j + 1) * BLOCK],
                                rhs=qT[:, (i * H + h0) * BLOCK: (i * H + h0 + 2) * BLOCK],
                                start=True, stop=True,
                            )
                        nc.scalar.activation(pt[:, :(i + 1) * ncols], st[:, :(i + 1) * ncols],
                                             mybir.ActivationFunctionType.Exp, scale=SCALE)
                        for h4 in range(2):
                            h = h0 + h4
                            oa = oacc_pool.tile([128, D + 1], F32, tag="oacc",
                                                name=f"oacc_h{h}_i{i}")
                            for j in range(i + 1):
                                vslot = (kv * NBLK + j) * VST
                                nc.tensor.matmul(
                                    oa[:],
                                    lhsT=pt[:, j * ncols + h4 * BLOCK: j * ncols + (h4 + 1) * BLOCK],
                                    rhs=vb[:, vslot: vslot + D + 1],
                                    start=(j == 0), stop=(j == i),
                                )
                            rcp = small_pool.tile([128, 1], F32, tag="rcp")
                            nc.vector.reciprocal(rcp[:], oa[:, D: D + 1])
                            osb = osb_pool.tile([128, D], F32, tag="osb")
                            nc.vector.tensor_scalar_mul(osb[:], oa[:, 0: D], rcp[:])
                            nc.sync.dma_start(
                                o_d[i * BLOCK:(i + 1) * BLOCK, h * D:(h + 1) * D], osb[:])

                    # spread cache copies through the attention loop
                    if job_idx < len(cache_jobs):
                        dst, src = cache_jobs[job_idx]
                        nc.sync.dma_start(dst, src)
                        job_idx += 1
            # any leftover cache jobs
            while job_idx < len(cache_jobs):
                dst, src = cache_jobs[job_idx]
                nc.sync.dma_start(dst, src)
                job_idx += 1


_PROG = None


def build_program():
    global _PROG
    if _PROG is None:
        nc = bacc.Bacc("TRN2", target_bir_lowering=False, debug=False,
                       num_devices=N_CORES)
        with tile.TileContext(nc) as tc:
            _emit(nc, tc)
        nc.compile()
        _PROG = nc
    return _PROG


def make_in_maps(q, k, v, k_cache, v_cache):
    in_maps = []
    for c in range(N_CORES):
        sl = slice(c * S, (c + 1) * S)
        un = slice(T + c * UN_PER_CORE, T + (c + 1) * UN_PER_CORE)
        in_maps.append({
            "q": np.ascontiguousarray(q[sl]),
            "k": np.ascontiguousarray(k[sl]),
            "v": np.ascontiguousarray(v[sl]),
            "kc_un": np.ascontiguousarray(k_cache[un]),
            "vc_un": np.ascontiguousarray(v_cache[un]),
        })
    return in_maps


def _gather(results, k_cache, v_cache):
    o = np.concatenate([results[c]["o"] for c in range(N_CORES)], axis=0)
    kc = np.empty((NUM_SLOTS, KV, D), np.float32)
    vc = np.empty((NUM_SLOTS, KV, D), np.float32)
    for c in range(N_CORES):
        kc[c * S:(c + 1) * S] = results[c]["kc_out"][:S]
        vc[c * S:(c + 1) * S] = results[c]["vc_out"][:S]
        kc[T + c * UN_PER_CORE: T + (c + 1) * UN_PER_CORE] = results[c]["kc_out"][S:]
        vc[T + c * UN_PER_CORE: T + (c + 1) * UN_PER_CORE] = results[c]["vc_out"][S:]
    return o, kc, vc


def _numpy_fallback(q, k, v, k_cache, v_cache, slot_mapping, cu_seqlens_q):
    """Exact reference semantics in numpy, used only if inputs deviate from
    the hardcoded fast-path layout."""
    kc = k_cache.copy()
    vc = v_cache.copy()
    kc[slot_mapping] = k
    vc[slot_mapping] = v
    b = cu_seqlens_q.shape[0] - 1
    s = q.shape[0] // b
    qb = q.reshape(b, s, H, D)
    kb = np.repeat(k.reshape(b, s, KV, D), G, axis=2)
    vb_ = np.repeat(v.reshape(b, s, KV, D), G, axis=2)
    blk = np.arange(s) // BLOCK
    mask = blk[:, None] >= blk[None, :]
    scores = np.einsum("bqhd,bkhd->bhqk", qb * SCALE, kb)
    scores = np.where(mask[None, None], scores, np.finfo(np.float32).min)
    scores = scores - scores.max(-1, keepdims=True)
    p = np.exp(scores)
    p = p / p.sum(-1, keepdims=True)
    o = np.einsum("bhqk,bkhd->bqhd", p, vb_).astype(np.float32)
    return o.reshape(b * s, H * D), kc, vc


def kernel(q, k, v, k_cache, v_cache, slot_mapping, cu_seqlens_q, cu_seqlens_k,
           block_size):
    q = np.asarray(q, np.float32)
    k = np.asarray(k, np.float32)
    v = np.asarray(v, np.float32)
    k_cache = np.asarray(k_cache, np.float32)
    v_cache = np.asarray(v_cache, np.float32)
    slot_mapping = np.asarray(slot_mapping)
    cu_seqlens_q = np.asarray(cu_seqlens_q)
    cu_seqlens_k = np.asarray(cu_seqlens_k)

    fast = (
        q.shape == (T, H, D) and k.shape == (T, KV, D) and v.shape == (T, KV, D)
        and k_cache.shape == (NUM_SLOTS, KV, D) and v_cache.shape == (NUM_SLOTS, KV, D)
        and int(np.asarray(block_size)) == BLOCK
        and np.array_equal(slot_mapping, np.arange(T, dtype=slot_mapping.dtype))
        and np.array_equal(cu_seqlens_q, np.arange(B + 1, dtype=cu_seqlens_q.dtype) * S)
        and np.array_equal(cu_seqlens_k, np.arange(B + 1, dtype=cu_seqlens_k.dtype) * S)
    )
    if not fast:
        return _numpy_fallback(q, k, v, k_cache, v_cache, slot_mapping, cu_seqlens_q)

    nc = build_program()
    in_maps = make_in_maps(q, k, v, k_cache, v_cache)
    res = bass_utils.run_bass_kernel_spmd(nc, in_maps, core_ids=list(range(N_CORES)))
    return _gather(res.results, k_cache, v_cache)


if __name__ == "__main__":
    rng = np.random.default_rng(0)
    q = rng.standard_normal((T, H, D), dtype=np.float32)
    k = rng.standard_normal((T, KV, D), dtype=np.float32)
    v = rng.standard_normal((T, KV, D), dtype=np.float32)
    kc = np.zeros((NUM_SLOTS, KV, D), np.float32)
    vc = np.zeros((NUM_SLOTS, KV, D), np.float32)
    sm = np.arange(T, dtype=np.int32)
    cu = np.arange(B + 1, dtype=np.int32) * S
    o, kcn, vcn = kernel(q=q, k=k, v=v, k_cache=kc, v_cache=vc, slot_mapping=sm,
                         cu_seqlens_q=cu, cu_seqlens_k=cu, block_size=128)
    oref, kref, vref = _numpy_fallback(q, k, v, kc, vc, sm, cu)
    print("o relerr:", np.abs(o - oref).max() / np.abs(oref).max())
    print("kc equal:", np.array_equal(kcn, kref), "vc equal:", np.array_equal(vcn, vref))
